# revision 30
# baseline (speedup 1.0000x reference)
"""Trainium2 Bass kernel for nn_DART: MADE masked-MLP + Normal log-prob +
HMM-style log-domain chain, data-parallel over batch on 8 NeuronCores.

Self-contained: hardcodes shapes from the problem spec
(B=1024, D=256, H=512, A=4, K=32). kernel(**inputs) takes full inputs,
shards batch across 8 cores, runs one SPMD Bass kernel, gathers outputs.

Per-core algorithm (B_local = 128 on 128 partitions):
  1. Front MLP in transposed layout: hT[H, B] via PE matmuls; bias+relu
     fused into the PSUM->SBUF copy on ScalarE.
  2. Output layer theta[B, 8192] streamed in 16 chunks of 512. Weights are
     split bf16 hi/lo; theta = hh@Wh + hh@Wl + hl@Wh (error ~2^-17), bias
     via K=1 ones-vector matmuls (bf16 hi/lo, exact to 2^-18).
  3. Phase A in 4 groups of 4 chunks: std=exp(t1)+0.01, chain matrices
     q = lpaC - 0.5*z^2 - ln(std), per-matrix max offsets, P0=exp(q-max).
  4. Scaled-probability binary tree over 256 4x4 matrices per sample:
     prob-domain products with per-level max rescaling; offsets accumulate
     separately (each node contributes exactly once). Levels 1-2 run per
     128-leaf half (left half overlaps the streaming phase), levels 3+ on
     the merged 64 matrices. k-sums are two strided tensor_tensor adds
     (2-port reads) instead of tensor_reduce (1 elem/cycle).
  5. log_p = ln(P_root[0,0]) + sum(all offsets). lpaC masks the boundary
     matrices (row 0 / col 0 only), keeping the chain well-scaled.
"""
import sys
sys.path.insert(0, "/opt/trn_rl_repo")

import numpy as np

B_FULL = 1024
NCORES = 8
B = B_FULL // NCORES          # 128, = SBUF partitions
D, H, A = 256, 512, 4
K = 2 * A * A                 # 32
DK = D * K                    # 8192
NCHUNK = 16                   # theta chunks of 512 (= 16 d-values each)
CW = DK // NCHUNK             # 512
DPC = D // NCHUNK             # 16 d per chunk
NGRP = 8                      # phase-A groups (2 chunks each)
GW = DK // NGRP               # 2048 theta cols per group
DPG = D // NGRP               # 64 d per group
C_CONST = float(0.5 * np.log(2.0 * np.pi))
NEG = np.float32(-1e30)

_cached = {}


def _host_prep(W0, W1, W2, Wout, b0, b1, b2, bout, ulpa):
    import ml_dtypes
    bf16 = ml_dtypes.bfloat16
    in_deg = np.arange(D)
    hid_deg = np.arange(H) % (D - 1)
    out_deg = np.arange(D) - 1
    m0 = (hid_deg[:, None] >= in_deg[None, :]).astype(np.float32)
    mh = (hid_deg[:, None] >= hid_deg[None, :]).astype(np.float32)
    ml = (out_deg[:, None] >= hid_deg[None, :]).astype(np.float32)
    ml = np.repeat(ml, K, axis=0)
    # transposed masked weights, rearranged so each partition's DMA read is
    # one contiguous run: [p, kc, cols]
    w0t = (m0 * W0).T.reshape(2, 128, H).transpose(1, 0, 2)      # [128, 2, 512]
    w1t = (mh * W1).T.reshape(4, 128, H).transpose(1, 0, 2)      # [128, 4, 512]
    w2t = (mh * W2).T.reshape(4, 128, H).transpose(1, 0, 2)      # [128, 4, 512]
    woutt = (ml * Wout).T                                        # [512, 8192]
    wa = woutt.reshape(4, 128, NCHUNK, CW).transpose(2, 1, 0, 3)  # [16,128,4,512]
    wout_hi = wa.astype(bf16)
    wout_lo = (wa - wout_hi.astype(np.float32)).astype(bf16)
    b0a = b0.reshape(4, 128).T
    b1a = b1.reshape(4, 128).T
    b2a = b2.reshape(4, 128).T
    bout_hi = bout.reshape(1, DK).astype(bf16)
    bout_lo = (bout.reshape(1, DK) - bout_hi.astype(np.float32)).astype(bf16)
    u = ulpa[0, :, 0, :].astype(np.float32)        # [255, 4]
    umax = u.max(-1, keepdims=True)
    lpa = u - (np.log(np.sum(np.exp(u - umax), -1, keepdims=True)) + umax)
    lpac = np.zeros((D, A, A), np.float32)
    lpac[:255] = lpa[:, None, :] - C_CONST
    lpac[255] = -C_CONST
    lpac[0, 1:, :] = NEG               # first term: row 0 only
    lpac[255, :, 1:] = NEG             # last term: col 0 only
    # per-d constant leaf offset: exp(lpac - od - hz2 - ln) <= e^|min ln std|
    # stays bounded, so the per-matrix rowmax reduce/subtract on device is
    # unnecessary; the offset sum returns as one host constant.
    od = lpac.reshape(D, 16).max(-1)
    lpac = (lpac.reshape(D, 16) - od[:, None]).reshape(D, A, A)
    off_const = float(od.sum(dtype=np.float64))
    c = np.ascontiguousarray
    return dict(w0t=c(w0t.astype(np.float32)), w1t=c(w1t.astype(np.float32)),
                w2t=c(w2t.astype(np.float32)), wouth=c(wout_hi),
                woutl=c(wout_lo), b0=c(b0a.astype(np.float32)),
                b1=c(b1a.astype(np.float32)), b2=c(b2a.astype(np.float32)),
                bouth=c(bout_hi), boutl=c(bout_lo),
                lpac=c(lpac.reshape(-1))), off_const


def _split_excess_waits(nc, mybir, cap=1):
    """This walrus build rejects instructions carrying more than `cap` sem
    waits; move the excess onto same-engine NoOps emitted just before."""
    for f in nc.m.functions:
        for bb in f.blocks:
            new_insts = []
            for ins in bb.instructions:
                si = ins.sync_info
                if si is not None and si.on_wait and len(si.on_wait) > cap:
                    waits = list(si.on_wait)
                    extra, keep = waits[:-cap], waits[-cap:]
                    for i, w in enumerate(extra):
                        new_insts.append(mybir.InstNoOp(
                            name=f"{ins.name}_splitw{i}",
                            engine=ins.engine,
                            bass_nofuse=True,
                            sync_info=mybir.SyncInfo(on_wait=[w], on_update=[]),
                        ))
                    si.on_wait = keep
                new_insts.append(ins)
            bb.instructions[:] = new_insts


def _build(split_waits=True):
    import concourse.bass as bass
    import concourse.tile as tile
    from concourse import mybir

    FP = mybir.dt.float32
    BF = mybir.dt.bfloat16
    AF = mybir.ActivationFunctionType
    OP = mybir.AluOpType
    AX = mybir.AxisListType

    nc = bass.Bass()
    xt_d = nc.dram_tensor("xt", [B, 2, B], FP, kind="ExternalInput")
    x_d = nc.dram_tensor("x", [B, D], FP, kind="ExternalInput")
    w0t_d = nc.dram_tensor("w0t", [B, 2, H], FP, kind="ExternalInput")
    w1t_d = nc.dram_tensor("w1t", [B, 4, H], FP, kind="ExternalInput")
    w2t_d = nc.dram_tensor("w2t", [B, 4, H], FP, kind="ExternalInput")
    wouth_d = nc.dram_tensor("wouth", [NCHUNK, B, 4, CW], BF, kind="ExternalInput")
    woutl_d = nc.dram_tensor("woutl", [NCHUNK, B, 4, CW], BF, kind="ExternalInput")
    b0_d = nc.dram_tensor("b0", [B, 4], FP, kind="ExternalInput")
    b1_d = nc.dram_tensor("b1", [B, 4], FP, kind="ExternalInput")
    b2_d = nc.dram_tensor("b2", [B, 4], FP, kind="ExternalInput")
    bouth_d = nc.dram_tensor("bouth", [1, DK], BF, kind="ExternalInput")
    boutl_d = nc.dram_tensor("boutl", [1, DK], BF, kind="ExternalInput")
    lpac_d = nc.dram_tensor("lpac", [D * 16], FP, kind="ExternalInput")
    theta_d = nc.dram_tensor("theta", [B, DK], FP, kind="ExternalOutput")
    logp_d = nc.dram_tensor("logp", [B, 1], FP, kind="ExternalOutput")

    with tile.TileContext(nc) as tc:
        from contextlib import ExitStack
        with ExitStack() as ctx:
            consts = ctx.enter_context(tc.tile_pool(name="consts", bufs=1))
            hpool = ctx.enter_context(tc.tile_pool(name="h", bufs=1))
            wopool = ctx.enter_context(tc.tile_pool(name="wo", bufs=3))
            bopool = ctx.enter_context(tc.tile_pool(name="bo", bufs=2))
            ppool = ctx.enter_context(tc.tile_pool(name="pa", bufs=1))
            big = ctx.enter_context(tc.tile_pool(name="big", bufs=1))
            tpool = ctx.enter_context(tc.tile_pool(name="tree", bufs=1))
            psf = ctx.enter_context(tc.tile_pool(name="psf", bufs=4, space="PSUM"))
            psw = ctx.enter_context(tc.tile_pool(name="psw", bufs=4, space="PSUM"))

            # ---- constants in ----
            xt_t = consts.tile([B, 2, B], FP)
            nc.sync.dma_start(out=xt_t, in_=xt_d[:])
            x_t = consts.tile([B, D], FP)
            nc.sync.dma_start(out=x_t, in_=x_d[:])
            w0t_t = consts.tile([B, 2, H], FP)
            nc.sync.dma_start(out=w0t_t, in_=w0t_d[:])
            w1t_t = consts.tile([B, 4, H], FP)
            nc.sync.dma_start(out=w1t_t, in_=w1t_d[:])
            w2t_t = consts.tile([B, 4, H], FP)
            nc.sync.dma_start(out=w2t_t, in_=w2t_d[:])
            b0_t = consts.tile([B, 4], FP)
            nc.sync.dma_start(out=b0_t, in_=b0_d[:])
            b1_t = consts.tile([B, 4], FP)
            nc.sync.dma_start(out=b1_t, in_=b1_d[:])
            b2_t = consts.tile([B, 4], FP)
            nc.sync.dma_start(out=b2_t, in_=b2_d[:])
            lpac_t = consts.tile([B, D * 16], FP)
            nc.sync.dma_start(out=lpac_t,
                              in_=lpac_d[:].unsqueeze(0).broadcast_to((B, D * 16)))
            ones_t = consts.tile([1, B], BF)
            nc.vector.memset(ones_t, 1.0)
            c01_t = consts.tile([B, 1], FP)
            nc.vector.memset(c01_t, 0.01)

            # offset accumulator: [0:256) leaf rowmax, [256:510) level ln(m)
            off_t = big.tile([B, 512], FP)
            nc.vector.memset(off_t, 0.0)

            # ---- front MLP (transposed: hT[H, B]) ----
            def front_layer(w_t, nk, rhs_t, bias_t, out_t):
                for oc in range(4):
                    ps = psf.tile([128, B], FP, tag="psf")
                    for kc in range(nk):
                        nc.tensor.matmul(ps,
                                         lhsT=w_t[:, kc, oc * 128:(oc + 1) * 128],
                                         rhs=rhs_t[:, kc, :],
                                         start=(kc == 0), stop=(kc == nk - 1))
                    nc.scalar.activation(out_t[:, oc, :], ps, AF.Relu,
                                         bias=bias_t[:, oc:oc + 1], scale=1.0)

            with tc.high_priority():
                h1_t = hpool.tile([B, 4, B], FP)
                front_layer(w0t_t, 2, xt_t, b0_t, h1_t)
                h2_t = hpool.tile([B, 4, B], FP)
                front_layer(w1t_t, 4, h1_t, b1_t, h2_t)
                h3_t = hpool.tile([B, 4, B], FP)
                front_layer(w2t_t, 4, h2_t, b2_t, h3_t)

            # h3 -> bf16 hi + lo for the 3-term split-precision matmul
            hh_t = hpool.tile([B, 4, B], BF)
            nc.scalar.copy(hh_t, h3_t)
            hl_t = hpool.tile([B, 4, B], BF)
            nc.vector.tensor_tensor(out=hl_t, in0=h3_t, in1=hh_t, op=OP.subtract)

            # ---- output layer (16 chunks) + phase A (4 groups) ----
            theta_t = big.tile([B, DK], FP)
            # leaf matrices, one tile per quarter (64 d each) for fine-grained
            # scheduling; tree reads them via explicit APs.
            p0_t = [big.tile([B, D * 2], FP, tag=f"p0{g}", name=f"p0{g}")
                    for g in range(NGRP)]

            def phase_a(g):
                lo = g * GW                          # theta col offset
                do = g * DPG                         # d offset
                th = theta_t[:, lo:lo + GW].rearrange(
                    "p (d t e) -> p d t e", t=2, e=16)
                mu_v = th[:, :, 0, :]                # [B, 64, 16]
                t1_v = th[:, :, 1, :]
                std_c = ppool.tile([B, DPG, 16], FP, tag="std")
                nc.scalar.activation(std_c, t1_v, AF.Exp)
                ln_c = ppool.tile([B, DPG, 16], FP, tag="ln")
                # ln(std) = Ln(exp(t1) + 0.01) via the activation bias
                nc.scalar.activation(ln_c, std_c, AF.Ln,
                                     bias=c01_t[:, 0:1], scale=1.0)
                rstd_c = ppool.tile([B, DPG, 16], FP, tag="rstd")
                nc.scalar.activation(rstd_c, ln_c, AF.Exp, scale=-1.0)
                z_c = ppool.tile([B, DPG, 16], FP, tag="z")
                nc.vector.tensor_tensor(
                    out=z_c,
                    in0=x_t[:, do:do + DPG].unsqueeze(2).broadcast_to((B, DPG, 16)),
                    in1=mu_v, op=OP.subtract)
                nc.vector.tensor_tensor(out=z_c, in0=z_c, in1=rstd_c, op=OP.mult)
                hz2_c = ppool.tile([B, DPG, 16], FP, tag="hz2")
                nc.scalar.activation(hz2_c, z_c, AF.Square,
                                     scale=float(np.sqrt(0.5)))
                q_c = ppool.tile([B, DPG, 16], FP, tag="q")
                nc.vector.tensor_tensor(
                    out=q_c,
                    in0=lpac_t[:, do * 16:(do + DPG) * 16]
                        .rearrange("p (d e) -> p d e", e=16),
                    in1=hz2_c, op=OP.subtract)
                nc.vector.tensor_tensor(out=q_c, in0=q_c, in1=ln_c, op=OP.subtract)
                nc.scalar.activation(
                    p0_t[g][:].rearrange("p (d e) -> p d e", e=16), q_c, AF.Exp)

            for nck in range(NCHUNK):
                lo = nck * CW
                woh_t = wopool.tile([B, 4, CW], BF, tag="woh")
                nc.sync.dma_start(out=woh_t, in_=wouth_d[nck])
                wol_t = wopool.tile([B, 4, CW], BF, tag="wol")
                nc.sync.dma_start(out=wol_t, in_=woutl_d[nck])
                boh_t = bopool.tile([1, CW], BF, tag="boh")
                nc.sync.dma_start(out=boh_t, in_=bouth_d[0:1, lo:lo + CW])
                bol_t = bopool.tile([1, CW], BF, tag="bol")
                nc.sync.dma_start(out=bol_t, in_=boutl_d[0:1, lo:lo + CW])
                ps = psw.tile([128, CW], FP, tag="psw")
                nc.tensor.matmul(ps, lhsT=ones_t, rhs=boh_t,
                                 start=True, stop=False)
                nc.tensor.matmul(ps, lhsT=ones_t, rhs=bol_t,
                                 start=False, stop=False)
                for kc in range(4):
                    nc.tensor.matmul(ps, lhsT=hh_t[:, kc, :], rhs=woh_t[:, kc, :],
                                     start=False, stop=False)
                    nc.tensor.matmul(ps, lhsT=hh_t[:, kc, :], rhs=wol_t[:, kc, :],
                                     start=False, stop=False)
                    nc.tensor.matmul(ps, lhsT=hl_t[:, kc, :], rhs=woh_t[:, kc, :],
                                     start=False, stop=(kc == 3))
                nc.scalar.copy(theta_t[:, lo:lo + CW], ps)
                if nck % 2 == 1:
                    phase_a(nck // 2)



            nc.sync.dma_start(out=theta_d[:], in_=theta_t)

            # ---- scaled-probability pairwise tree ----
            # pair product: T[p,i,j,k] = L[i,k] * R[k,j] (4 TT muls), then
            # k-sum via two strided TT adds, then max-rescale.
            def pair_level(cur_aps, n, T_t, S_out, U_t):
                # cur_aps: list of (ap, n_mats) covering the level's input
                # matrices in order; total pairs = n.
                done = 0
                for ap, nm in cur_aps:
                    npairs = nm // 2
                    part = list(ap.ap[0])
                    Tv = T_t[:, done * 64:(done + npairs) * 64].rearrange(
                        "p (n i j k) -> p n i j k", i=4, j=4, k=4)
                    for i in range(4):
                        in0 = bass.AP(tensor=ap.tensor, offset=ap.offset + i * 4,
                                      ap=[part, [32, npairs], [0, 4], [1, 4]])
                        in1 = bass.AP(tensor=ap.tensor, offset=ap.offset + 16,
                                      ap=[part, [32, npairs], [1, 4], [4, 4]])
                        nc.vector.tensor_tensor(out=Tv[:, :, i, :, :], in0=in0,
                                                in1=in1, op=OP.mult)
                    done += npairs
                # k-sum: U[a, kp] = T[a, 2kp] + T[a, 2kp+1]; S[a] = U[a,0]+U[a,1]
                tpart = list(T_t.ap[0])
                nc.vector.tensor_tensor(
                    out=U_t[:, :n * 32],
                    in0=bass.AP(tensor=T_t.tensor, offset=T_t.offset,
                                ap=[tpart, [4, n * 16], [2, 2]]),
                    in1=bass.AP(tensor=T_t.tensor, offset=T_t.offset + 1,
                                ap=[tpart, [4, n * 16], [2, 2]]),
                    op=OP.add)
                upart = list(U_t.ap[0])
                nc.vector.tensor_tensor(
                    out=S_out,
                    in0=bass.AP(tensor=U_t.tensor, offset=U_t.offset,
                                ap=[upart, [2, n * 16]]),
                    in1=bass.AP(tensor=U_t.tensor, offset=U_t.offset + 1,
                                ap=[upart, [2, n * 16]]),
                    op=OP.add)

            def renorm(S_t, n, ooff, tag):
                m_t = tpool.tile([B, n], FP, tag=f"m{tag}")
                nc.vector.tensor_reduce(
                    out=m_t, in_=S_t.rearrange("p (n e) -> p n e", e=16),
                    axis=AX.X, op=OP.max)
                nc.vector.tensor_scalar_max(out=m_t, in0=m_t, scalar1=1e-30)
                nc.scalar.activation(off_t[:, ooff:ooff + n], m_t, AF.Ln)
                rm_t = tpool.tile([B, n], FP, tag=f"r{tag}")
                nc.vector.reciprocal(rm_t, m_t)
                Sv = S_t.rearrange("p (n e) -> p n e", e=16)
                nc.vector.tensor_tensor(
                    out=Sv, in0=Sv,
                    in1=rm_t.unsqueeze(2).broadcast_to((B, n, 16)), op=OP.mult)

            T_t = tpool.tile([B, DK // 2], FP, tag="T")
            U_t = tpool.tile([B, DK // 4], FP, tag="U")
            S2ab = tpool.tile([B, 1024], FP, tag="S2ab")  # 64 mats after L2
            ooff = 256
            for q in range(4):
                # per-quarter L1+L2 (64 leaves -> 16 mats). L1 skips renorm:
                # leaves are max-normalized (<=1), so L1 products are <=4 and
                # L2 products stay well inside fp32 range; L2 renormalizes.
                S1 = tpool.tile([B, 32 * 16], FP, tag=f"S1{q}", name=f"S1{q}")
                pair_level([(p0_t[2 * q][:], 32), (p0_t[2 * q + 1][:], 32)],
                           32, T_t, S1, U_t)
                pair_level([(S1[:], 32)], 16, T_t,
                           S2ab[:, q * 256:(q + 1) * 256], U_t)
                renorm(S2ab[:, q * 256:(q + 1) * 256], 16, ooff, f"2{q}")
                ooff += 16
            # merged levels: 64 -> 32 -> 16 -> 8 -> 4 -> 2 -> 1
            cur = S2ab
            n_mats = 64
            lev = 3
            while n_mats > 1:
                n = n_mats // 2
                S_t = tpool.tile([B, n * 16], FP, tag=f"S{lev}")
                pair_level([(cur[:], n_mats)], n, T_t, S_t, U_t)
                if n > 8:
                    renorm(S_t, n, ooff, str(lev))
                    ooff += n
                cur = S_t
                n_mats = n
                lev += 1
            # log_p = ln(root[0,0]) + sum(offsets); masking makes root have
            # only element (0,0) nonzero.
            osum_t = tpool.tile([B, 1], FP, tag="osum")
            nc.vector.tensor_reduce(out=osum_t, in_=off_t, axis=AX.X, op=OP.add)
            lnp_t = tpool.tile([B, 1], FP, tag="lnp")
            nc.scalar.activation(lnp_t, cur[:, 0:1], AF.Ln)
            res_t = tpool.tile([B, 1], FP, tag="res")
            nc.vector.tensor_tensor(out=res_t, in0=lnp_t, in1=osum_t, op=OP.add)
            nc.sync.dma_start(out=logp_d[:], in_=res_t)

    if split_waits:
        _split_excess_waits(nc, mybir)
    return nc


def kernel(x, W0, b0, W1, b1, W2, b2, Wout, bout, ulpa):
    from concourse.bass_utils import run_bass_kernel_spmd

    x = np.ascontiguousarray(np.asarray(x, np.float32))
    shared, off_const = _host_prep(
        np.asarray(W0, np.float32), np.asarray(W1, np.float32),
        np.asarray(W2, np.float32), np.asarray(Wout, np.float32),
        np.asarray(b0, np.float32), np.asarray(b1, np.float32),
        np.asarray(b2, np.float32), np.asarray(bout, np.float32),
        np.asarray(ulpa, np.float32))

    if "nc" not in _cached:
        _cached["nc"] = _build()
    nc = _cached["nc"]

    in_maps = []
    for c in range(NCORES):
        xs = x[c * B:(c + 1) * B]
        xt = np.ascontiguousarray(
            xs.T.reshape(2, 128, B).transpose(1, 0, 2))
        in_maps.append({**shared, "x": np.ascontiguousarray(xs), "xt": xt})

    import os
    trace = bool(os.environ.get("BASS_KERNEL_TRACE"))
    res = run_bass_kernel_spmd(nc, in_maps, core_ids=list(range(NCORES)),
                               trace=trace)
    _cached["last_results"] = res

    theta = np.concatenate([r["theta"] for r in res.results], axis=0)
    logp = np.concatenate([r["logp"] for r in res.results], axis=0)
    logp = logp + np.float32(off_const)
    return logp.reshape(B_FULL, 1, 1), theta.reshape(B_FULL, D, 2, A, A)


# revision 31
# speedup vs baseline: 1.0557x; 1.0557x over previous
"""Trainium2 Bass kernel for nn_DART: MADE masked-MLP + Normal log-prob +
HMM-style log-domain chain, data-parallel over batch on 8 NeuronCores.

Self-contained: hardcodes shapes from the problem spec
(B=1024, D=256, H=512, A=4, K=32). kernel(**inputs) takes full inputs,
shards batch across 8 cores, runs one SPMD Bass kernel, gathers outputs.

Per-core algorithm (B_local = 128 on 128 partitions):
  1. Front MLP in transposed layout: hT[H, B] via PE matmuls; bias+relu
     fused into the PSUM->SBUF copy on ScalarE.
  2. Output layer theta[B, 8192] streamed in 16 chunks of 512. Weights are
     split bf16 hi/lo; theta = hh@Wh + hh@Wl + hl@Wh (error ~2^-17), bias
     via K=1 ones-vector matmuls (bf16 hi/lo, exact to 2^-18).
  3. Phase A in 4 groups of 4 chunks: std=exp(t1)+0.01, chain matrices
     q = lpaC - 0.5*z^2 - ln(std), per-matrix max offsets, P0=exp(q-max).
  4. Scaled-probability binary tree over 256 4x4 matrices per sample:
     prob-domain products with per-level max rescaling; offsets accumulate
     separately (each node contributes exactly once). Levels 1-2 run per
     128-leaf half (left half overlaps the streaming phase), levels 3+ on
     the merged 64 matrices. k-sums are two strided tensor_tensor adds
     (2-port reads) instead of tensor_reduce (1 elem/cycle).
  5. log_p = ln(P_root[0,0]) + sum(all offsets). lpaC masks the boundary
     matrices (row 0 / col 0 only), keeping the chain well-scaled.
"""
import sys
sys.path.insert(0, "/opt/trn_rl_repo")

import numpy as np

B_FULL = 1024
NCORES = 8
B = B_FULL // NCORES          # 128, = SBUF partitions
D, H, A = 256, 512, 4
K = 2 * A * A                 # 32
DK = D * K                    # 8192
NCHUNK = 16                   # theta chunks of 512 (= 16 d-values each)
CW = DK // NCHUNK             # 512
DPC = D // NCHUNK             # 16 d per chunk
NGRP = 8                      # phase-A groups (2 chunks each)
GW = DK // NGRP               # 2048 theta cols per group
DPG = D // NGRP               # 64 d per group
C_CONST = float(0.5 * np.log(2.0 * np.pi))
NEG = np.float32(-1e30)

_cached = {}


def _host_prep(W0, W1, W2, Wout, b0, b1, b2, bout, ulpa):
    import ml_dtypes
    bf16 = ml_dtypes.bfloat16
    in_deg = np.arange(D)
    hid_deg = np.arange(H) % (D - 1)
    out_deg = np.arange(D) - 1
    m0 = (hid_deg[:, None] >= in_deg[None, :]).astype(np.float32)
    mh = (hid_deg[:, None] >= hid_deg[None, :]).astype(np.float32)
    ml = (out_deg[:, None] >= hid_deg[None, :]).astype(np.float32)
    ml = np.repeat(ml, K, axis=0)
    # transposed masked weights, rearranged so each partition's DMA read is
    # one contiguous run: [p, kc, cols]
    w0t = (m0 * W0).T.reshape(2, 128, H).transpose(1, 0, 2)      # [128, 2, 512]
    w1t = (mh * W1).T.reshape(4, 128, H).transpose(1, 0, 2)      # [128, 4, 512]
    w2t = (mh * W2).T.reshape(4, 128, H).transpose(1, 0, 2)      # [128, 4, 512]
    woutt = (ml * Wout).T                                        # [512, 8192]
    wa = woutt.reshape(4, 128, NCHUNK, CW).transpose(2, 1, 0, 3)  # [16,128,4,512]
    wout_hi = wa.astype(bf16)
    wout_lo = (wa - wout_hi.astype(np.float32)).astype(bf16)
    b0a = b0.reshape(4, 128).T
    b1a = b1.reshape(4, 128).T
    b2a = b2.reshape(4, 128).T
    bout_hi = bout.reshape(1, DK).astype(bf16)
    bout_lo = (bout.reshape(1, DK) - bout_hi.astype(np.float32)).astype(bf16)
    u = ulpa[0, :, 0, :].astype(np.float32)        # [255, 4]
    umax = u.max(-1, keepdims=True)
    lpa = u - (np.log(np.sum(np.exp(u - umax), -1, keepdims=True)) + umax)
    lpac = np.zeros((D, A, A), np.float32)
    lpac[:255] = lpa[:, None, :] - C_CONST
    lpac[255] = -C_CONST
    lpac[0, 1:, :] = NEG               # first term: row 0 only
    lpac[255, :, 1:] = NEG             # last term: col 0 only
    # per-d constant leaf offset: exp(lpac - od - hz2 - ln) <= e^|min ln std|
    # stays bounded, so the per-matrix rowmax reduce/subtract on device is
    # unnecessary; the offset sum returns as one host constant.
    od = lpac.reshape(D, 16).max(-1)
    lpac = (lpac.reshape(D, 16) - od[:, None]).reshape(D, A, A)
    off_const = float(od.sum(dtype=np.float64))
    c = np.ascontiguousarray
    return dict(w0t=c(w0t.astype(np.float32)), w1t=c(w1t.astype(np.float32)),
                w2t=c(w2t.astype(np.float32)), wouth=c(wout_hi),
                woutl=c(wout_lo), b0=c(b0a.astype(np.float32)),
                b1=c(b1a.astype(np.float32)), b2=c(b2a.astype(np.float32)),
                bouth=c(bout_hi), boutl=c(bout_lo),
                lpac=c(lpac.reshape(-1))), off_const


def _split_excess_waits(nc, mybir, cap=1):
    """This walrus build rejects instructions carrying more than `cap` sem
    waits; move the excess onto same-engine NoOps emitted just before."""
    for f in nc.m.functions:
        for bb in f.blocks:
            new_insts = []
            for ins in bb.instructions:
                si = ins.sync_info
                if si is not None and si.on_wait and len(si.on_wait) > cap:
                    waits = list(si.on_wait)
                    extra, keep = waits[:-cap], waits[-cap:]
                    for i, w in enumerate(extra):
                        new_insts.append(mybir.InstNoOp(
                            name=f"{ins.name}_splitw{i}",
                            engine=ins.engine,
                            bass_nofuse=True,
                            sync_info=mybir.SyncInfo(on_wait=[w], on_update=[]),
                        ))
                    si.on_wait = keep
                new_insts.append(ins)
            bb.instructions[:] = new_insts


def _build(split_waits=True):
    import concourse.bass as bass
    import concourse.tile as tile
    from concourse import mybir

    FP = mybir.dt.float32
    BF = mybir.dt.bfloat16
    AF = mybir.ActivationFunctionType
    OP = mybir.AluOpType
    AX = mybir.AxisListType

    nc = bass.Bass()
    xt_d = nc.dram_tensor("xt", [B, 2, B], FP, kind="ExternalInput")
    x_d = nc.dram_tensor("x", [B, D], FP, kind="ExternalInput")
    w0t_d = nc.dram_tensor("w0t", [B, 2, H], FP, kind="ExternalInput")
    w1t_d = nc.dram_tensor("w1t", [B, 4, H], FP, kind="ExternalInput")
    w2t_d = nc.dram_tensor("w2t", [B, 4, H], FP, kind="ExternalInput")
    wouth_d = nc.dram_tensor("wouth", [NCHUNK, B, 4, CW], BF, kind="ExternalInput")
    woutl_d = nc.dram_tensor("woutl", [NCHUNK, B, 4, CW], BF, kind="ExternalInput")
    b0_d = nc.dram_tensor("b0", [B, 4], FP, kind="ExternalInput")
    b1_d = nc.dram_tensor("b1", [B, 4], FP, kind="ExternalInput")
    b2_d = nc.dram_tensor("b2", [B, 4], FP, kind="ExternalInput")
    bouth_d = nc.dram_tensor("bouth", [1, DK], BF, kind="ExternalInput")
    boutl_d = nc.dram_tensor("boutl", [1, DK], BF, kind="ExternalInput")
    lpac_d = nc.dram_tensor("lpac", [D * 16], FP, kind="ExternalInput")
    theta_d = nc.dram_tensor("theta", [B, DK], FP, kind="ExternalOutput")
    logp_d = nc.dram_tensor("logp", [B, 1], FP, kind="ExternalOutput")

    with tile.TileContext(nc) as tc:
        from contextlib import ExitStack
        with ExitStack() as ctx:
            consts = ctx.enter_context(tc.tile_pool(name="consts", bufs=1))
            hpool = ctx.enter_context(tc.tile_pool(name="h", bufs=1))
            wopool = ctx.enter_context(tc.tile_pool(name="wo", bufs=3))
            bopool = ctx.enter_context(tc.tile_pool(name="bo", bufs=2))
            ppool = ctx.enter_context(tc.tile_pool(name="pa", bufs=1))
            big = ctx.enter_context(tc.tile_pool(name="big", bufs=1))
            tpool = ctx.enter_context(tc.tile_pool(name="tree", bufs=1))
            psf = ctx.enter_context(tc.tile_pool(name="psf", bufs=4, space="PSUM"))
            psw = ctx.enter_context(tc.tile_pool(name="psw", bufs=4, space="PSUM"))

            # ---- constants in ----
            xt_t = consts.tile([B, 2, B], FP)
            nc.sync.dma_start(out=xt_t, in_=xt_d[:])
            x_t = consts.tile([B, D], FP)
            nc.sync.dma_start(out=x_t, in_=x_d[:])
            w0t_t = consts.tile([B, 2, H], FP)
            nc.sync.dma_start(out=w0t_t, in_=w0t_d[:])
            w1t_t = consts.tile([B, 4, H], FP)
            nc.sync.dma_start(out=w1t_t, in_=w1t_d[:])
            w2t_t = consts.tile([B, 4, H], FP)
            nc.sync.dma_start(out=w2t_t, in_=w2t_d[:])
            b0_t = consts.tile([B, 4], FP)
            nc.sync.dma_start(out=b0_t, in_=b0_d[:])
            b1_t = consts.tile([B, 4], FP)
            nc.sync.dma_start(out=b1_t, in_=b1_d[:])
            b2_t = consts.tile([B, 4], FP)
            nc.sync.dma_start(out=b2_t, in_=b2_d[:])
            lpac_t = consts.tile([B, D * 16], FP)
            nc.sync.dma_start(out=lpac_t,
                              in_=lpac_d[:].unsqueeze(0).broadcast_to((B, D * 16)))
            ones_t = consts.tile([1, B], BF)
            nc.vector.memset(ones_t, 1.0)
            c01_t = consts.tile([B, 1], FP)
            nc.vector.memset(c01_t, 0.01)

            # offset accumulator: [0:256) leaf rowmax, [256:510) level ln(m)
            off_t = big.tile([B, 512], FP)
            nc.vector.memset(off_t, 0.0)

            # ---- front MLP (transposed: hT[H, B]) ----
            def front_layer(w_t, nk, rhs_t, bias_t, out_t):
                for oc in range(4):
                    ps = psf.tile([128, B], FP, tag="psf")
                    for kc in range(nk):
                        nc.tensor.matmul(ps,
                                         lhsT=w_t[:, kc, oc * 128:(oc + 1) * 128],
                                         rhs=rhs_t[:, kc, :],
                                         start=(kc == 0), stop=(kc == nk - 1))
                    nc.scalar.activation(out_t[:, oc, :], ps, AF.Relu,
                                         bias=bias_t[:, oc:oc + 1], scale=1.0)

            with tc.high_priority():
                h1_t = hpool.tile([B, 4, B], FP)
                front_layer(w0t_t, 2, xt_t, b0_t, h1_t)
                h2_t = hpool.tile([B, 4, B], FP)
                front_layer(w1t_t, 4, h1_t, b1_t, h2_t)
                h3_t = hpool.tile([B, 4, B], FP)
                front_layer(w2t_t, 4, h2_t, b2_t, h3_t)

            # h3 -> bf16 hi + lo for the 3-term split-precision matmul
            hh_t = hpool.tile([B, 4, B], BF)
            nc.vector.tensor_copy(hh_t, h3_t)
            hl_t = hpool.tile([B, 4, B], BF)
            nc.vector.tensor_tensor(out=hl_t, in0=h3_t, in1=hh_t, op=OP.subtract)

            # ---- output layer (16 chunks) + phase A (4 groups) ----
            theta_t = big.tile([B, DK], FP)
            # leaf matrices, one tile per quarter (64 d each) for fine-grained
            # scheduling; tree reads them via explicit APs.
            p0_t = [big.tile([B, D * 2], FP, tag=f"p0{g}", name=f"p0{g}")
                    for g in range(NGRP)]

            def phase_a(g):
                lo = g * GW                          # theta col offset
                do = g * DPG                         # d offset
                th = theta_t[:, lo:lo + GW].rearrange(
                    "p (d t e) -> p d t e", t=2, e=16)
                mu_v = th[:, :, 0, :]                # [B, 64, 16]
                t1_v = th[:, :, 1, :]
                std_c = ppool.tile([B, DPG, 16], FP, tag="std")
                nc.scalar.activation(std_c, t1_v, AF.Exp)
                nc.scalar.activation(std_c, std_c, AF.Identity,
                                     bias=c01_t[:, 0:1], scale=1.0)
                ln_c = ppool.tile([B, DPG, 16], FP, tag="ln")
                nc.scalar.activation(ln_c, std_c, AF.Ln)
                rstd_c = ppool.tile([B, DPG, 16], FP, tag="rstd")
                nc.scalar.activation(rstd_c, ln_c, AF.Exp, scale=-1.0)
                z_c = ppool.tile([B, DPG, 16], FP, tag="z")
                nc.vector.tensor_tensor(
                    out=z_c,
                    in0=x_t[:, do:do + DPG].unsqueeze(2).broadcast_to((B, DPG, 16)),
                    in1=mu_v, op=OP.subtract)
                nc.vector.tensor_tensor(out=z_c, in0=z_c, in1=rstd_c, op=OP.mult)
                hz2_c = ppool.tile([B, DPG, 16], FP, tag="hz2")
                nc.scalar.activation(hz2_c, z_c, AF.Square,
                                     scale=float(np.sqrt(0.5)))
                q_c = ppool.tile([B, DPG, 16], FP, tag="q")
                nc.vector.tensor_tensor(
                    out=q_c,
                    in0=lpac_t[:, do * 16:(do + DPG) * 16]
                        .rearrange("p (d e) -> p d e", e=16),
                    in1=hz2_c, op=OP.subtract)
                nc.vector.tensor_tensor(out=q_c, in0=q_c, in1=ln_c, op=OP.subtract)
                nc.scalar.activation(
                    p0_t[g][:].rearrange("p (d e) -> p d e", e=16), q_c, AF.Exp)

            for nck in range(NCHUNK):
                lo = nck * CW
                woh_t = wopool.tile([B, 4, CW], BF, tag="woh")
                nc.sync.dma_start(out=woh_t, in_=wouth_d[nck])
                wol_t = wopool.tile([B, 4, CW], BF, tag="wol")
                nc.sync.dma_start(out=wol_t, in_=woutl_d[nck])
                boh_t = bopool.tile([1, CW], BF, tag="boh")
                nc.sync.dma_start(out=boh_t, in_=bouth_d[0:1, lo:lo + CW])
                bol_t = bopool.tile([1, CW], BF, tag="bol")
                nc.sync.dma_start(out=bol_t, in_=boutl_d[0:1, lo:lo + CW])
                ps = psw.tile([128, CW], FP, tag="psw")
                nc.tensor.matmul(ps, lhsT=ones_t, rhs=boh_t,
                                 start=True, stop=False)
                nc.tensor.matmul(ps, lhsT=ones_t, rhs=bol_t,
                                 start=False, stop=False)
                for kc in range(4):
                    nc.tensor.matmul(ps, lhsT=hh_t[:, kc, :], rhs=woh_t[:, kc, :],
                                     start=False, stop=False)
                    nc.tensor.matmul(ps, lhsT=hh_t[:, kc, :], rhs=wol_t[:, kc, :],
                                     start=False, stop=False)
                    nc.tensor.matmul(ps, lhsT=hl_t[:, kc, :], rhs=woh_t[:, kc, :],
                                     start=False, stop=(kc == 3))
                nc.scalar.copy(theta_t[:, lo:lo + CW], ps)
                if nck % 2 == 1:
                    phase_a(nck // 2)



            nc.sync.dma_start(out=theta_d[:], in_=theta_t)

            # ---- scaled-probability pairwise tree ----
            # pair product: T[p,i,j,k] = L[i,k] * R[k,j] (4 TT muls), then
            # k-sum via two strided TT adds, then max-rescale.
            def pair_level(cur_aps, n, T_t, S_out, U_t):
                # cur_aps: list of (ap, n_mats) covering the level's input
                # matrices in order; total pairs = n.
                done = 0
                for ap, nm in cur_aps:
                    npairs = nm // 2
                    part = list(ap.ap[0])
                    Tv = T_t[:, done * 64:(done + npairs) * 64].rearrange(
                        "p (n i j k) -> p n i j k", i=4, j=4, k=4)
                    for i in range(4):
                        in0 = bass.AP(tensor=ap.tensor, offset=ap.offset + i * 4,
                                      ap=[part, [32, npairs], [0, 4], [1, 4]])
                        in1 = bass.AP(tensor=ap.tensor, offset=ap.offset + 16,
                                      ap=[part, [32, npairs], [1, 4], [4, 4]])
                        nc.vector.tensor_tensor(out=Tv[:, :, i, :, :], in0=in0,
                                                in1=in1, op=OP.mult)
                    done += npairs
                # k-sum: U[a, kp] = T[a, 2kp] + T[a, 2kp+1]; S[a] = U[a,0]+U[a,1]
                tpart = list(T_t.ap[0])
                nc.vector.tensor_tensor(
                    out=U_t[:, :n * 32],
                    in0=bass.AP(tensor=T_t.tensor, offset=T_t.offset,
                                ap=[tpart, [4, n * 16], [2, 2]]),
                    in1=bass.AP(tensor=T_t.tensor, offset=T_t.offset + 1,
                                ap=[tpart, [4, n * 16], [2, 2]]),
                    op=OP.add)
                upart = list(U_t.ap[0])
                nc.vector.tensor_tensor(
                    out=S_out,
                    in0=bass.AP(tensor=U_t.tensor, offset=U_t.offset,
                                ap=[upart, [2, n * 16]]),
                    in1=bass.AP(tensor=U_t.tensor, offset=U_t.offset + 1,
                                ap=[upart, [2, n * 16]]),
                    op=OP.add)

            def renorm(S_t, n, ooff, tag):
                m_t = tpool.tile([B, n], FP, tag=f"m{tag}")
                nc.vector.tensor_reduce(
                    out=m_t, in_=S_t.rearrange("p (n e) -> p n e", e=16),
                    axis=AX.X, op=OP.max)
                nc.vector.tensor_scalar_max(out=m_t, in0=m_t, scalar1=1e-30)
                nc.scalar.activation(off_t[:, ooff:ooff + n], m_t, AF.Ln)
                rm_t = tpool.tile([B, n], FP, tag=f"r{tag}")
                nc.vector.reciprocal(rm_t, m_t)
                Sv = S_t.rearrange("p (n e) -> p n e", e=16)
                nc.vector.tensor_tensor(
                    out=Sv, in0=Sv,
                    in1=rm_t.unsqueeze(2).broadcast_to((B, n, 16)), op=OP.mult)

            T_t = tpool.tile([B, DK // 2], FP, tag="T")
            U_t = tpool.tile([B, DK // 4], FP, tag="U")
            S2ab = tpool.tile([B, 1024], FP, tag="S2ab")  # 64 mats after L2
            ooff = 256
            for q in range(4):
                # per-quarter L1+L2 (64 leaves -> 16 mats). L1 skips renorm:
                # leaves are max-normalized (<=1), so L1 products are <=4 and
                # L2 products stay well inside fp32 range; L2 renormalizes.
                S1 = tpool.tile([B, 32 * 16], FP, tag=f"S1{q}", name=f"S1{q}")
                pair_level([(p0_t[2 * q][:], 32), (p0_t[2 * q + 1][:], 32)],
                           32, T_t, S1, U_t)
                pair_level([(S1[:], 32)], 16, T_t,
                           S2ab[:, q * 256:(q + 1) * 256], U_t)
                renorm(S2ab[:, q * 256:(q + 1) * 256], 16, ooff, f"2{q}")
                ooff += 16
            # merged levels: 64 -> 32 -> 16 -> 8 -> 4 -> 2 -> 1
            cur = S2ab
            n_mats = 64
            lev = 3
            while n_mats > 1:
                n = n_mats // 2
                S_t = tpool.tile([B, n * 16], FP, tag=f"S{lev}")
                pair_level([(cur[:], n_mats)], n, T_t, S_t, U_t)
                if n > 8:
                    renorm(S_t, n, ooff, str(lev))
                    ooff += n
                cur = S_t
                n_mats = n
                lev += 1
            # log_p = ln(root[0,0]) + sum(offsets); masking makes root have
            # only element (0,0) nonzero.
            osum_t = tpool.tile([B, 1], FP, tag="osum")
            nc.vector.tensor_reduce(out=osum_t, in_=off_t, axis=AX.X, op=OP.add)
            lnp_t = tpool.tile([B, 1], FP, tag="lnp")
            nc.scalar.activation(lnp_t, cur[:, 0:1], AF.Ln)
            res_t = tpool.tile([B, 1], FP, tag="res")
            nc.vector.tensor_tensor(out=res_t, in0=lnp_t, in1=osum_t, op=OP.add)
            nc.sync.dma_start(out=logp_d[:], in_=res_t)

    if split_waits:
        _split_excess_waits(nc, mybir)
    return nc


def kernel(x, W0, b0, W1, b1, W2, b2, Wout, bout, ulpa):
    from concourse.bass_utils import run_bass_kernel_spmd

    x = np.ascontiguousarray(np.asarray(x, np.float32))
    shared, off_const = _host_prep(
        np.asarray(W0, np.float32), np.asarray(W1, np.float32),
        np.asarray(W2, np.float32), np.asarray(Wout, np.float32),
        np.asarray(b0, np.float32), np.asarray(b1, np.float32),
        np.asarray(b2, np.float32), np.asarray(bout, np.float32),
        np.asarray(ulpa, np.float32))

    if "nc" not in _cached:
        _cached["nc"] = _build()
    nc = _cached["nc"]

    in_maps = []
    for c in range(NCORES):
        xs = x[c * B:(c + 1) * B]
        xt = np.ascontiguousarray(
            xs.T.reshape(2, 128, B).transpose(1, 0, 2))
        in_maps.append({**shared, "x": np.ascontiguousarray(xs), "xt": xt})

    import os
    trace = bool(os.environ.get("BASS_KERNEL_TRACE"))
    res = run_bass_kernel_spmd(nc, in_maps, core_ids=list(range(NCORES)),
                               trace=trace)
    _cached["last_results"] = res

    theta = np.concatenate([r["theta"] for r in res.results], axis=0)
    logp = np.concatenate([r["logp"] for r in res.results], axis=0)
    logp = logp + np.float32(off_const)
    return logp.reshape(B_FULL, 1, 1), theta.reshape(B_FULL, D, 2, A, A)
